# revision 1
# baseline (speedup 1.0000x reference)
"""Trainium2 Bass kernel for multiscale K-Planes embedding lookup + MLP.

Computation (see reference): for each of 2M points (x0,x1,x2,t):
  per scale s (4 scales) and plane p (3 planes): bilinear-sample
  plane[p] at (x_p, t), multiply the 3 planes' 16-ch features,
  concat over scales -> [N,64], then 3-layer MLP -> [N,64].

Strategy:
  - Data parallel: shard points over 8 cores, replicate tables/weights.
  - Host precomputes "delta form" corner tables: one 64-float row per
    grid cell = [v00, dy, dx, dxy] x 16ch, so bilinear =
    A = v00 + wx*dx ; B = dy + wx*dxy ; res = A + wy*B.
  - Points are y-binned on host (buckets of 16 t-rows) so each block's
    table window fits int16 indices for the GPSIMD dma_gather.
  - Device: ACT computes grid coords, DVE computes floor/weights/int16
    local row indices, Pool dma_gather fetches one 256B row per
    (point, plane, scale), DVE lerps + plane products, PE runs the MLP.
"""

import math
import numpy as np
from contextlib import ExitStack

import concourse.bass as bass
import concourse.bacc as bacc
import concourse.mybir as mybir
import concourse.tile as tile
from concourse import library_config
from concourse.masks import make_identity

FP = mybir.dt.float32
I16 = mybir.dt.int16

H = 150
WS = (64, 128, 256, 512)
NP = 3            # planes per scale
NS = 4            # scales
ROW = 64          # floats per table row (4 corners x 16 ch)
NCORES = 8
YB = 16           # y-bucket height (t rows per bucket)
NBKT = (H - 1 + YB - 1) // YB   # 10 buckets

# tuning
K = 16            # point-cols per partition per block (block = 128*K pts)
MM_N = 512        # MLP moving-dim chunk (points per matmul)
REPLICATE_IDX = True   # replicate int16 idxs across the 8 partition groups


def wlo_of_bucket(b):
    """Window start row (with -1 slack) per bucket; clamped >= 0."""
    return max(0, b * YB - 1)


def win_rows(b):
    """Rows in the window incl +-1 slack."""
    lo = wlo_of_bucket(b)
    hi = min(H - 1, (b + 1) * YB + 1)    # exclusive iy0 bound + slack
    return hi - lo


# ---------------------------------------------------------------------------
# device program
# ---------------------------------------------------------------------------

def build_program(block_buckets, k: int = K, num_devices: int = 1, dbg: bool = False):
    """block_buckets: bucket id per block (len = nb)."""
    nc = bacc.Bacc("TRN2", target_bir_lowering=False, debug=False,
                   enable_asserts=False, num_devices=num_devices)

    KC = k
    nb = len(block_buckets)
    L = nb * 128 * KC
    nrows = [(H - 1) * NP * (w - 1) for w in WS]

    coords = nc.dram_tensor("coords", [nb, 128, 13 * KC], FP, kind="ExternalInput").ap()
    tabs = [nc.dram_tensor(f"tab{s}", [nrows[s], ROW], FP, kind="ExternalInput").ap()
            for s in range(NS)]
    consts_d = nc.dram_tensor("consts", [128, 37], FP, kind="ExternalInput").ap()
    w0t_d = nc.dram_tensor("w0t", [64, 128], FP, kind="ExternalInput").ap()
    w1t_d = nc.dram_tensor("w1t", [128, 128], FP, kind="ExternalInput").ap()
    w2t_d = nc.dram_tensor("w2t", [128, 64], FP, kind="ExternalInput").ap()
    b0_d = nc.dram_tensor("b0c", [128, 1], FP, kind="ExternalInput").ap()
    b1_d = nc.dram_tensor("b1c", [128, 1], FP, kind="ExternalInput").ap()
    b2_d = nc.dram_tensor("b2r", [128, 64], FP, kind="ExternalInput").ap()
    out_d = nc.dram_tensor("out", [L, 64], FP, kind="ExternalOutput").ap()
    if dbg:
        dbg_wcat = nc.dram_tensor("dbg_wcat", [nb, 128, 13 * k], FP,
                                  kind="ExternalOutput").ap()
        dbg_offs = nc.dram_tensor("dbg_offs", [nb, 128, 12 * k], FP,
                                  kind="ExternalOutput").ap()
        dbg_g0 = nc.dram_tensor("dbg_g0", [nb, 128, 3 * k * ROW], FP,
                                kind="ExternalOutput").ap()
        dbg_feats = nc.dram_tensor("dbg_feats", [nb, 128, k * 64], FP,
                                   kind="ExternalOutput").ap()

    # per-(scale,plane) affine consts for the 12 x-streams + 1 t-stream
    alphas = [0.5 * (WS[s] - 1) for s in range(NS) for _ in range(NP)] + [0.5 * (H - 1)]

    with tile.TileContext(nc) as tc:
        with ExitStack() as ctx:
            cpool = ctx.enter_context(tc.tile_pool(name="cpool", bufs=1))
            ppool = ctx.enter_context(tc.tile_pool(name="ppool", bufs=2))
            gpool = ctx.enter_context(tc.tile_pool(name="gpool", bufs=3))
            ipool = ctx.enter_context(tc.tile_pool(name="ipool", bufs=2))
            fpool = ctx.enter_context(tc.tile_pool(name="fpool", bufs=2))
            mpool = ctx.enter_context(tc.tile_pool(name="mpool", bufs=2))
            qpool = ctx.enter_context(tc.tile_pool(name="qpool", bufs=2, space="PSUM"))

            nc.gpsimd.load_library(library_config.mlp)

            # ---- constants (loaded once) ----
            ident = cpool.tile([128, 128], FP)
            make_identity(nc, ident)
            cns = cpool.tile([128, 37], FP)
            nc.sync.dma_start(cns, consts_d)
            w0t = cpool.tile([64, 128], FP)
            nc.sync.dma_start(w0t, w0t_d)
            w1t = cpool.tile([128, 128], FP)
            nc.sync.dma_start(w1t, w1t_d)
            w2t = cpool.tile([128, 64], FP)
            nc.sync.dma_start(w2t, w2t_d)
            b0 = cpool.tile([128, 1], FP)
            nc.sync.dma_start(b0, b0_d)
            b1 = cpool.tile([128, 1], FP)
            nc.sync.dma_start(b1, b1_d)
            b2r = cpool.tile([128, 64], FP)
            nc.sync.dma_start(b2r, b2_d)

            plb = cns[:, 0:12]     # plane id per (s,pl): 0,1,2 repeating
            wm1 = cns[:, 12:24]    # W_s-1
            wmax = cns[:, 24:37]   # W_s-2 (12 x-blocks), 148 (y block)

            for blk in range(nb):
                bkt = block_buckets[blk]
                # ---- phase A: coords -> weights + int16 local row indices --
                ct = ppool.tile([128, 13 * KC], FP)
                nc.sync.dma_start(ct, coords[blk])

                ixf = ppool.tile([128, 13 * KC], FP)
                for j in range(13):
                    a = alphas[j]
                    nc.scalar.activation(
                        ixf[:, j * KC:(j + 1) * KC], ct[:, j * KC:(j + 1) * KC],
                        mybir.ActivationFunctionType.Copy, bias=a, scale=a)

                ci = ppool.tile([128, 13 * KC], I16)
                # int16 covers ix (<512) and iy (<150)
                nc.vector.tensor_copy(ci, ixf)
                cfa = ppool.tile([128, 13 * KC], FP)
                nc.vector.tensor_copy(cfa, ci)
                # fixup in case the cast rounded up (cfa > ixf)
                neg = ppool.tile([128, 13 * KC], FP)
                nc.vector.tensor_tensor(out=neg, in0=cfa, in1=ixf,
                                        op=mybir.AluOpType.is_gt)
                cf2 = ppool.tile([128, 13 * KC], FP)
                nc.vector.tensor_tensor(out=cf2, in0=cfa, in1=neg,
                                        op=mybir.AluOpType.subtract)
                # clamp to <= W-2 (x) / 148 (y)
                cf3 = ppool.tile([128, 13 * KC], FP)
                wmaxb = wmax.unsqueeze(-1).to_broadcast([128, 13, KC])
                nc.vector.tensor_tensor(
                    out=cf3.rearrange("p (a c) -> p a c", a=13),
                    in0=cf2.rearrange("p (a c) -> p a c", a=13),
                    in1=wmaxb, op=mybir.AluOpType.min)
                wcat = ppool.tile([128, 13 * KC], FP)
                nc.vector.tensor_tensor(out=wcat, in0=ixf, in1=cf3,
                                        op=mybir.AluOpType.subtract)

                # local row index = (iy*3 + pl)*(W-1) + ix - wlo_s*3*(W-1)
                iy3 = ppool.tile([128, KC], FP)
                nc.vector.tensor_scalar(out=iy3, in0=cf3[:, 12 * KC:13 * KC],
                                        scalar1=3.0, scalar2=None,
                                        op0=mybir.AluOpType.mult)
                ypc = ppool.tile([128, 12 * KC], FP)
                iyb = iy3.unsqueeze(1).to_broadcast([128, 12, KC])
                plbb = plb.unsqueeze(-1).to_broadcast([128, 12, KC])
                nc.vector.tensor_tensor(
                    out=ypc.rearrange("p (a c) -> p a c", a=12),
                    in0=iyb, in1=plbb, op=mybir.AluOpType.add)
                tmp12 = ppool.tile([128, 12 * KC], FP)
                wm1b = wm1.unsqueeze(-1).to_broadcast([128, 12, KC])
                nc.vector.tensor_tensor(
                    out=tmp12.rearrange("p (a c) -> p a c", a=12),
                    in0=ypc.rearrange("p (a c) -> p a c", a=12),
                    in1=wm1b, op=mybir.AluOpType.mult)
                idxf = ppool.tile([128, 12 * KC], FP)
                nc.vector.tensor_tensor(out=idxf, in0=tmp12,
                                        in1=cf3[:, 0:12 * KC],
                                        op=mybir.AluOpType.add)
                # subtract per-scale window base
                for s in range(NS):
                    base = float(wlo_of_bucket(bkt) * NP * (WS[s] - 1))
                    nc.vector.tensor_scalar(
                        out=idxf[:, s * 3 * KC:(s + 1) * 3 * KC],
                        in0=idxf[:, s * 3 * KC:(s + 1) * 3 * KC],
                        scalar1=-base, scalar2=None,
                        op0=mybir.AluOpType.add)
                if dbg:
                    nc.sync.dma_start(dbg_wcat[blk], wcat)
                    nc.sync.dma_start(dbg_offs[blk], idxf)

                # relayout to dma_gather idx wrap: idx j at [j%16, j//16];
                # j = (c*8 + p//16)*16 + p%16  ->  idx16[q, c*8+h] = idxf[h*16+q, c]
                # DVE needs partition starts at multiples of 32, so odd-h
                # groups read from a 16-partition-rotated copy.
                idxsh = ppool.tile([128, 12 * KC], FP)
                nc.sync.dma_start(idxsh[0:112, :], idxf[16:128, :])
                idx16 = ppool.tile([128, 12 * KC * 8], I16)
                i16v = idx16.rearrange("p (c h) -> p c h", h=8)
                for h in range(8):
                    src = idxf if h % 2 == 0 else idxsh
                    base_p = (h // 2) * 32
                    nc.vector.tensor_copy(
                        out=i16v[0:16, :, h],
                        in_=src[base_p:base_p + 16, :])
                if REPLICATE_IDX:
                    for grp in range(1, 8):
                        nc.sync.dma_start(
                            idx16[grp * 16:(grp + 1) * 16, :], idx16[0:16, :])

                # ---- per scale: gather + interpolate ----
                feats = fpool.tile([128, KC * 64], FP)
                featsv = feats.rearrange("p (c i) -> p c i", i=64)
                for s in range(NS):
                    base = wlo_of_bucket(bkt) * NP * (WS[s] - 1)
                    wrows = win_rows(bkt) * NP * (WS[s] - 1)
                    g = gpool.tile([128, 3 * KC * ROW], FP, tag="g")
                    nc.gpsimd.dma_gather(
                        out_ap=g.rearrange("p (c i) -> p c i", i=ROW),
                        in_ap=tabs[s][base:base + wrows],
                        idxs_ap=idx16[:, s * 3 * KC * 8:(s + 1) * 3 * KC * 8],
                        num_idxs=3 * KC * 128,
                        num_idxs_reg=3 * KC * 128,
                        elem_size=ROW,
                        single_packet=False)
                    if dbg and s == 0:
                        nc.sync.dma_start(dbg_g0[blk], g)
                    g4 = g.rearrange("p (pl c i) -> p pl c i", pl=3, i=ROW)
                    wx3 = (wcat[:, s * 3 * KC:(s + 1) * 3 * KC]
                           .rearrange("p (pl c) -> p pl c", pl=3)
                           .unsqueeze(-1))
                    wy3 = (wcat[:, 12 * KC:13 * KC]
                           .unsqueeze(1).unsqueeze(-1))

                    m = ipool.tile([128, 3 * KC * 32], FP)
                    m4 = m.rearrange("p (pl c i) -> p pl c i", pl=3, i=32)
                    nc.vector.tensor_tensor(
                        out=m4, in0=g4[:, :, :, 32:64],
                        in1=wx3.to_broadcast([128, 3, KC, 32]),
                        op=mybir.AluOpType.mult)
                    sab = ipool.tile([128, 3 * KC * 32], FP)
                    s4 = sab.rearrange("p (pl c i) -> p pl c i", pl=3, i=32)
                    nc.vector.tensor_tensor(
                        out=s4, in0=g4[:, :, :, 0:32], in1=m4,
                        op=mybir.AluOpType.add)
                    my = ipool.tile([128, 3 * KC * 16], FP)
                    my4 = my.rearrange("p (pl c i) -> p pl c i", pl=3, i=16)
                    nc.vector.tensor_tensor(
                        out=my4, in0=s4[:, :, :, 16:32],
                        in1=wy3.to_broadcast([128, 3, KC, 16]),
                        op=mybir.AluOpType.mult)
                    res = ipool.tile([128, 3 * KC * 16], FP)
                    res4 = res.rearrange("p (pl c i) -> p pl c i", pl=3, i=16)
                    nc.vector.tensor_tensor(
                        out=res4, in0=s4[:, :, :, 0:16], in1=my4,
                        op=mybir.AluOpType.add)
                    # product over the 3 planes
                    r3 = res.rearrange("p (pl x) -> p pl x", pl=3)
                    pp = ipool.tile([128, KC * 16], FP)
                    nc.vector.tensor_tensor(out=pp, in0=r3[:, 0], in1=r3[:, 1],
                                            op=mybir.AluOpType.mult)
                    nc.vector.tensor_tensor(
                        out=featsv[:, :, s * 16:(s + 1) * 16],
                        in0=pp.rearrange("p (c i) -> p c i", i=16),
                        in1=r3[:, 2].rearrange("p (c i) -> p c i", i=16),
                        op=mybir.AluOpType.mult)

                # ---- MLP ----
                if dbg:
                    nc.sync.dma_start(dbg_feats[blk], feats)
                outt = fpool.tile([128, KC * 64], FP)
                nchunk = (KC * 128) // MM_N      # chunks of MM_N points
                kper = MM_N // 128               # point-cols per chunk
                for cc in range(nchunk):
                    ftp = qpool.tile([64, MM_N], FP, space="PSUM", tag="ftp")
                    for j in range(kper):
                        kk = cc * kper + j
                        nc.tensor.transpose(
                            out=ftp[:, j * 128:(j + 1) * 128],
                            in_=featsv[:, kk, :], identity=ident)
                    fts = mpool.tile([64, MM_N], FP)
                    nc.scalar.activation(fts, ftp,
                                         mybir.ActivationFunctionType.Copy)
                    p0 = qpool.tile([128, MM_N], FP, space="PSUM", tag="p0")
                    nc.tensor.matmul(out=p0, lhsT=w0t, rhs=fts,
                                     start=True, stop=True)
                    h0 = mpool.tile([128, MM_N], FP)
                    nc.scalar.activation(h0, p0,
                                         mybir.ActivationFunctionType.Relu,
                                         bias=b0[:, 0:1])
                    p1 = qpool.tile([128, MM_N], FP, space="PSUM", tag="p1")
                    nc.tensor.matmul(out=p1, lhsT=w1t, rhs=h0,
                                     start=True, stop=True)
                    h1 = mpool.tile([128, MM_N], FP)
                    nc.scalar.activation(h1, p1,
                                         mybir.ActivationFunctionType.Relu,
                                         bias=b1[:, 0:1])
                    p2 = qpool.tile([128, kper * 64], FP, space="PSUM", tag="p2")
                    for j in range(kper):
                        nc.tensor.matmul(out=p2[:, j * 64:(j + 1) * 64],
                                         lhsT=h1[:, j * 128:(j + 1) * 128],
                                         rhs=w2t, start=True, stop=True)
                    for j in range(kper):
                        kk = cc * kper + j
                        nc.vector.tensor_tensor(
                            out=outt[:, kk * 64:(kk + 1) * 64],
                            in0=p2[:, j * 64:(j + 1) * 64], in1=b2r,
                            op=mybir.AluOpType.add)

                nc.sync.dma_start(
                    out_d[blk * 128 * KC:(blk + 1) * 128 * KC]
                    .rearrange("(p c) f -> p (c f)", p=128),
                    outt)

    nc.compile()
    return nc


# ---------------------------------------------------------------------------
# host-side data prep
# ---------------------------------------------------------------------------

def make_tables(planes_list):
    """[3,16,150,W] fp32 -> delta-form table [(H-1)*3*(W-1), 64];
    row index = (iy*3 + pl)*(W-1) + ix."""
    tabs = []
    for P in planes_list:
        Pt = np.ascontiguousarray(P.transpose(0, 2, 3, 1))  # [3,150,W,16]
        v00 = Pt[:, :-1, :-1]
        v01 = Pt[:, :-1, 1:]
        v10 = Pt[:, 1:, :-1]
        v11 = Pt[:, 1:, 1:]
        dx = v01 - v00
        dy = v10 - v00
        dxy = v11 - v10 - v01 + v00
        row = np.concatenate([v00, dy, dx, dxy], axis=-1)   # [3,149,W-1,64]
        row = row.transpose(1, 0, 2, 3)                     # [149,3,W-1,64]
        tabs.append(np.ascontiguousarray(row.reshape(-1, 64), dtype=np.float32))
    return tabs


def make_consts():
    c = np.zeros((128, 37), np.float32)
    for s in range(NS):
        for pl in range(NP):
            j = s * NP + pl
            c[:, j] = pl
            c[:, 12 + j] = WS[s] - 1
            c[:, 24 + j] = WS[s] - 2
    c[:, 36] = H - 2
    return c


def bucket_of_t(t):
    ay = np.float32(0.5 * (H - 1))
    iyf = t.astype(np.float32) * ay + ay
    iy0 = np.minimum(np.floor(iyf), H - 2).astype(np.int64)
    iy0 = np.maximum(iy0, 0)
    return np.minimum(iy0 // YB, NBKT - 1)


def make_coords(shard, nb, k):
    """shard [L,4] fp32 (already bucket-sorted+padded) -> [nb,128,13k]."""
    arr = shard.reshape(nb, 128, k, 4)
    c = np.empty((nb, 128, 13, k), np.float32)
    for s in range(NS):
        for pl in range(NP):
            c[:, :, s * NP + pl, :] = arr[:, :, :, pl]
    c[:, :, 12, :] = arr[:, :, :, 3]
    return np.ascontiguousarray(c.reshape(nb, 128, 13 * k))


def bucket_layout(pts, k):
    """Shard + y-bin all points. Returns per-core (coords, perm) plus the
    shared block_buckets (same for every core)."""
    n = pts.shape[0]
    percore = (n + NCORES - 1) // NCORES
    pb = 128 * k

    shards = []
    for c in range(NCORES):
        sh = pts[c * percore:(c + 1) * percore]
        shards.append(sh)

    order = [np.argsort(bucket_of_t(sh[:, 3]), kind="stable") for sh in shards]
    bkts = [bucket_of_t(sh[:, 3]) for sh in shards]
    counts = np.zeros((NCORES, NBKT), np.int64)
    for c in range(NCORES):
        for b in range(NBKT):
            counts[c, b] = int((bkts[c] == b).sum())
    nb_per_bucket = [int(math.ceil(counts[:, b].max() / pb)) for b in range(NBKT)]
    block_buckets = []
    for b in range(NBKT):
        block_buckets += [b] * nb_per_bucket[b]
    nb = len(block_buckets)
    L = nb * pb

    cores = []
    for c in range(NCORES):
        sh, od, bk = shards[c], order[c], bkts[c]
        coords_rows = np.zeros((L, 4), np.float32)
        perm = np.full(L, -1, np.int64)
        pos = 0
        for b in range(NBKT):
            sel = od[bk[od] == b]
            nrows = nb_per_bucket[b] * pb
            coords_rows[pos:pos + len(sel)] = sh[sel]
            # pad rows: t at bucket center so their indices stay in-window
            tpad = (b * YB + YB // 2) / (0.5 * (H - 1)) - 1.0
            if nrows > len(sel):
                coords_rows[pos + len(sel):pos + nrows, 3] = tpad
            perm[pos:pos + len(sel)] = sel
            pos += nrows
        cores.append((make_coords(coords_rows, nb, k), perm))
    return cores, block_buckets, percore


def host_inputs(pts, planes_list, w0, b0, w1, b1, w2, b2, k=K):
    tabs = make_tables(planes_list)
    consts = make_consts()
    shared = {
        "tab0": tabs[0], "tab1": tabs[1], "tab2": tabs[2], "tab3": tabs[3],
        "consts": consts,
        "w0t": np.ascontiguousarray(w0.T), "w1t": np.ascontiguousarray(w1.T),
        "w2t": np.ascontiguousarray(w2.T),
        "b0c": np.ascontiguousarray(b0.reshape(128, 1)),
        "b1c": np.ascontiguousarray(b1.reshape(128, 1)),
        "b2r": np.ascontiguousarray(np.broadcast_to(b2.reshape(1, 64), (128, 64))),
    }
    cores, block_buckets, percore = bucket_layout(pts, k)
    in_maps = [{**shared, "coords": co} for co, _ in cores]
    perms = [pm for _, pm in cores]
    return in_maps, perms, block_buckets, percore


# ---------------------------------------------------------------------------
# entry point
# ---------------------------------------------------------------------------

_CACHE = {}


def kernel(pts, planes_s0, planes_s1, planes_s2, planes_s3,
           w0, b0, w1, b1, w2, b2, _want_trace=False):
    from concourse.bass_utils import run_bass_kernel_spmd

    pts = np.asarray(pts, np.float32)
    planes = [np.asarray(p, np.float32)
              for p in (planes_s0, planes_s1, planes_s2, planes_s3)]
    in_maps, perms, block_buckets, percore = host_inputs(
        pts, planes,
        np.asarray(w0, np.float32), np.asarray(b0, np.float32),
        np.asarray(w1, np.float32), np.asarray(b1, np.float32),
        np.asarray(w2, np.float32), np.asarray(b2, np.float32))

    import time as _t
    key = (tuple(block_buckets), K)
    if key not in _CACHE:
        t0 = _t.time()
        print(f"[kernel] building program nb={len(block_buckets)}", flush=True)
        _CACHE[key] = build_program(block_buckets, K, num_devices=NCORES)
        print(f"[kernel] build done {_t.time()-t0:.1f}s", flush=True)
    nc = _CACHE[key]

    t0 = _t.time()
    print("[kernel] launching on 8 cores", flush=True)
    r = run_bass_kernel_spmd(nc, in_maps, core_ids=list(range(NCORES)),
                             trace=_want_trace)
    print(f"[kernel] run done {_t.time()-t0:.1f}s", flush=True)
    n = pts.shape[0]
    full = np.empty((n, 64), np.float32)
    for c in range(NCORES):
        dev = np.asarray(r.results[c]["out"])
        perm = perms[c]
        valid = perm >= 0
        base = c * percore
        full[base + perm[valid]] = dev[valid]
    if _want_trace:
        return full, r
    return full


if __name__ == "__main__":
    nc = build_program([0, 5], K)
    print("built ok")



# revision 2
# speedup vs baseline: 1.3658x; 1.3658x over previous
"""Trainium2 Bass kernel v3: quad-table gather for K-Planes lookup + MLP.

Key idea: the baseline gathers one 256B delta-form row per
(point, scale, plane) = 12 dma_gather indices per point, and the Q7
SWDGE descriptor generation (~8ns/index) dominates (90% of runtime).

v3 restructures the table so ONE gathered row serves all 4 scales of
one plane: row (iy, pl, j2) holds 12 corner COLUMNS [v(16ch), dy(16ch)]
(y-lerp delta form):
    s0: cols e0,e0+1,e0+2   e0 = (j2-3)//4
    s1: cols e1,e1+1,e1+2   e1 = (j2-1)//2
    s2: cols j2,j2+1
    s3: cols 2j2..2j2+3
where j2 = s2 cell of the point.  The candidate sets provably cover the
cells every scale needs.  x-interp becomes a hat-weighted sum over the
columns: res_s = sum_m relu(1-|z_s-m|) * (v_m + wy*dy_m), z_s = local
fractional coordinate (host-computed).  3 indices/point instead of 12.

bf16 table + bf16 interp + bf16 MLP (rel err ~7e-3 vs 2e-2 budget).
Host precomputes idx16 (wrapped+replicated int16) and the 13-stream
z/wy weights; the device does zero index math.
"""

import math
import numpy as np
import ml_dtypes
from contextlib import ExitStack

import concourse.bass as bass
import concourse.bacc as bacc
import concourse.mybir as mybir
import concourse.tile as tile
from concourse import library_config
from concourse.masks import make_identity

FP = mybir.dt.float32
BF = mybir.dt.bfloat16
I16 = mybir.dt.int16

H = 150
WS = (64, 128, 256, 512)
NP = 3
NS = 4
NCORES = 8
YB = 32
NBKT = (H - 2) // YB + 1          # iy0 in [0,148] -> 5 buckets
J2N = WS[2] - 1                   # 255 j2 values (cells 0..254 + clamp pad)
RPY = NP * J2N                    # rows per iy = 765
NCOL = 12                         # corner columns per row
RW = NCOL * 32                    # row elements (bf16): 384 = 768B
SLOT_S = [0, 0, 0, 1, 1, 1, 2, 2, 3, 3, 3, 3]   # scale of each col slot
SLOT_M = [0, 1, 2, 0, 1, 2, 0, 1, 0, 1, 2, 3]   # m offset of each col slot
SLOT_OFF = [0, 3, 6, 8]                          # first slot of each scale
SLOT_N = [3, 3, 2, 4]                            # cols per scale

K = 16            # point-cols per partition per block (block = 2048 pts)
MM_N = 512


def win_lo(b):
    return b * YB


def win_rows(b):
    return min(H - 1, (b + 1) * YB) - win_lo(b)   # <= 32


# ---------------------------------------------------------------------------
# device program
# ---------------------------------------------------------------------------

def build_program(block_buckets, k: int = K, num_devices: int = 1):
    nc = bacc.Bacc("TRN2", target_bir_lowering=False, debug=False,
                   enable_asserts=False, num_devices=num_devices)

    KC = k
    nb = len(block_buckets)
    L = nb * 128 * KC
    NIDX = NP * KC * 128

    wcat_d = nc.dram_tensor("wcat", [nb, 128, 13 * KC], FP, kind="ExternalInput").ap()
    idx_d = nc.dram_tensor("idx16", [nb, 128, NP * KC * 8], I16,
                           kind="ExternalInput").ap()
    tab_d = nc.dram_tensor("tab", [(H - 1) * RPY, RW], BF, kind="ExternalInput").ap()
    cns_d = nc.dram_tensor("cns", [128, NCOL], FP, kind="ExternalInput").ap()
    w0t_d = nc.dram_tensor("w0t", [64, 128], BF, kind="ExternalInput").ap()
    w1t_d = nc.dram_tensor("w1t", [128, 128], BF, kind="ExternalInput").ap()
    w2t_d = nc.dram_tensor("w2t", [128, 64], BF, kind="ExternalInput").ap()
    b0_d = nc.dram_tensor("b0c", [128, 1], FP, kind="ExternalInput").ap()
    b1_d = nc.dram_tensor("b1c", [128, 1], FP, kind="ExternalInput").ap()
    b2_d = nc.dram_tensor("b2r", [128, 64], FP, kind="ExternalInput").ap()
    out_d = nc.dram_tensor("out", [L, 64], FP, kind="ExternalOutput").ap()

    with tile.TileContext(nc) as tc:
        with ExitStack() as ctx:
            cpool = ctx.enter_context(tc.tile_pool(name="cpool", bufs=1))
            ppool = ctx.enter_context(tc.tile_pool(name="ppool", bufs=2))
            gpool = ctx.enter_context(tc.tile_pool(name="gpool", bufs=2))
            ipool = ctx.enter_context(tc.tile_pool(name="ipool", bufs=2))
            fpool = ctx.enter_context(tc.tile_pool(name="fpool", bufs=2))
            mpool = ctx.enter_context(tc.tile_pool(name="mpool", bufs=2))
            qpool = ctx.enter_context(tc.tile_pool(name="qpool", bufs=2, space="PSUM"))

            nc.gpsimd.load_library(library_config.mlp)

            identf = cpool.tile([128, 128], FP)
            make_identity(nc, identf)
            ident = cpool.tile([128, 128], BF)
            nc.vector.tensor_copy(ident, identf)
            cns = cpool.tile([128, NCOL], FP)
            nc.sync.dma_start(cns, cns_d)
            w0t = cpool.tile([64, 128], BF)
            nc.sync.dma_start(w0t, w0t_d)
            w1t = cpool.tile([128, 128], BF)
            nc.sync.dma_start(w1t, w1t_d)
            w2t = cpool.tile([128, 64], BF)
            nc.sync.dma_start(w2t, w2t_d)
            b0 = cpool.tile([128, 1], FP)
            nc.sync.dma_start(b0, b0_d)
            b1 = cpool.tile([128, 1], FP)
            nc.sync.dma_start(b1, b1_d)
            b2r = cpool.tile([128, 64], FP)
            nc.sync.dma_start(b2r, b2_d)

            for blk in range(nb):
                bkt = block_buckets[blk]
                wcat = ppool.tile([128, 13 * KC], FP)
                nc.sync.dma_start(wcat, wcat_d[blk])
                idx16 = ppool.tile([128, NP * KC * 8], I16)
                nc.sync.dma_start(idx16, idx_d[blk])

                # ---- gather: one row per (pt, plane) ----
                base = win_lo(bkt) * RPY
                wrows = win_rows(bkt) * RPY
                g = gpool.tile([128, NP * KC * RW], BF, tag="g")
                nc.gpsimd.dma_gather(
                    out_ap=g.rearrange("p (c i) -> p c i", i=RW),
                    in_ap=tab_d[base:base + wrows],
                    idxs_ap=idx16,
                    num_idxs=NIDX,
                    num_idxs_reg=NIDX,
                    elem_size=RW,
                    single_packet=False)

                # ---- hat weights (fp32 on DVE/ACT, cast to bf16) ----
                # zrep[p, pl, slot, kc] = z_s(pt) for slot's scale
                zrep = ppool.tile([128, NP * NCOL * KC], FP)
                zv = zrep.rearrange("p (pl sl c) -> p pl sl c", pl=NP, sl=NCOL)
                wv = wcat.rearrange("p (st c) -> p st c", st=13)
                for s in range(NS):
                    # in: streams s*3+pl, broadcast over this scale's slots
                    src = (wv[:, s * NP:(s + 1) * NP, :]
                           .unsqueeze(2)
                           .to_broadcast([128, NP, SLOT_N[s], KC]))
                    nc.vector.tensor_copy(
                        out=zv[:, :, SLOT_OFF[s]:SLOT_OFF[s] + SLOT_N[s], :],
                        in_=src)
                zm = ppool.tile([128, NP * NCOL * KC], FP)
                # in1[p, pl, sl, c] = cns[p, sl]
                nc.vector.tensor_tensor(
                    out=zm.rearrange("p (pl sl c) -> p pl sl c", pl=NP, sl=NCOL),
                    in0=zv,
                    in1=cns.unsqueeze(1).unsqueeze(3)
                        .to_broadcast([128, NP, NCOL, KC]),
                    op=mybir.AluOpType.subtract)
                azm = ppool.tile([128, NP * NCOL * KC], FP)
                nc.scalar.activation(azm, zm, mybir.ActivationFunctionType.Abs)
                hat = ppool.tile([128, NP * NCOL * KC], FP)
                nc.scalar.activation(hat, azm, mybir.ActivationFunctionType.Relu,
                                     bias=1.0, scale=-1.0)
                hd = ppool.tile([128, NP * NCOL * KC], FP)
                wyb = (wcat[:, 12 * KC:13 * KC].unsqueeze(1).unsqueeze(1)
                       .to_broadcast([128, NP, NCOL, KC]))
                nc.vector.tensor_tensor(
                    out=hd.rearrange("p (pl sl c) -> p pl sl c", pl=NP, sl=NCOL),
                    in0=hat.rearrange("p (pl sl c) -> p pl sl c", pl=NP, sl=NCOL),
                    in1=wyb, op=mybir.AluOpType.mult)
                # wh[p, pl, kc, slot, 2] bf16
                wh = ppool.tile([128, NP * KC * NCOL * 2], BF)
                whv = wh.rearrange("p (pl c sl h) -> p pl c sl h",
                                   pl=NP, c=KC, sl=NCOL)
                hatv = hat.rearrange("p (pl sl c) -> p pl c sl",
                                     pl=NP, sl=NCOL)
                hdv = hd.rearrange("p (pl sl c) -> p pl c sl",
                                   pl=NP, sl=NCOL)
                nc.vector.tensor_copy(out=whv[:, :, :, :, 0], in_=hatv)
                nc.vector.tensor_copy(out=whv[:, :, :, :, 1], in_=hdv)

                # ---- apply: g *= wh (broadcast over 16 channels) ----
                gv = g.rearrange("p (pl c sh ch) -> p pl c sh ch",
                                 pl=NP, c=KC, ch=16)
                whb = (wh.rearrange("p (pl c sh) -> p pl c sh", pl=NP, c=KC)
                       .unsqueeze(-1).to_broadcast([128, NP, KC, NCOL * 2, 16]))
                nc.vector.tensor_tensor(out=gv, in0=gv, in1=whb,
                                        op=mybir.AluOpType.mult)
                # fold v/dy halves: tcol = g[..., v] + g[..., dy]
                tcol = ipool.tile([128, NP * KC * NCOL * 16], BF)
                tv = tcol.rearrange("p (pl c sl ch) -> p pl c sl ch",
                                    pl=NP, c=KC, ch=16)
                g6 = g.rearrange("p (pl c sl h ch) -> p pl c sl h ch",
                                 pl=NP, c=KC, h=2, ch=16)
                nc.vector.tensor_tensor(out=tv, in0=g6[:, :, :, :, 0, :],
                                        in1=g6[:, :, :, :, 1, :],
                                        op=mybir.AluOpType.add)
                # ragged adds per scale -> res[p, pl, c, 64]
                res = ipool.tile([128, NP * KC * 64], BF)
                rv = res.rearrange("p (pl c f) -> p pl c f", pl=NP, c=KC)
                for s in range(NS):
                    o = SLOT_OFF[s]
                    dst = rv[:, :, :, s * 16:(s + 1) * 16]
                    nc.vector.tensor_tensor(
                        out=dst, in0=tv[:, :, :, o, :], in1=tv[:, :, :, o + 1, :],
                        op=mybir.AluOpType.add)
                    for m in range(2, SLOT_N[s]):
                        nc.vector.tensor_tensor(
                            out=dst, in0=dst, in1=tv[:, :, :, o + m, :],
                            op=mybir.AluOpType.add)
                # plane product -> feats [p, c, 64]
                pp = ipool.tile([128, KC * 64], BF)
                ppv = pp.rearrange("p (c f) -> p c f", f=64)
                nc.vector.tensor_tensor(out=ppv, in0=rv[:, 0],
                                        in1=rv[:, 1],
                                        op=mybir.AluOpType.mult)
                feats = fpool.tile([128, KC * 64], BF)
                nc.vector.tensor_tensor(out=feats.rearrange("p (c f) -> p c f", f=64),
                                        in0=ppv,
                                        in1=rv[:, 2],
                                        op=mybir.AluOpType.mult)
                featsv = feats.rearrange("p (c i) -> p c i", i=64)

                # ---- MLP (bf16 matmuls, fp32 psum) ----
                outt = fpool.tile([128, KC * 64], FP)
                nchunk = (KC * 128) // MM_N
                kper = MM_N // 128
                for cc in range(nchunk):
                    ftp = qpool.tile([64, MM_N], BF, space="PSUM", tag="ftp")
                    for j in range(kper):
                        kk = cc * kper + j
                        nc.tensor.transpose(
                            out=ftp[:, j * 128:(j + 1) * 128],
                            in_=featsv[:, kk, :], identity=ident)
                    fts = mpool.tile([64, MM_N], BF)
                    nc.scalar.activation(fts, ftp,
                                         mybir.ActivationFunctionType.Copy)
                    p0 = qpool.tile([128, MM_N], FP, space="PSUM", tag="p0")
                    nc.tensor.matmul(out=p0, lhsT=w0t, rhs=fts,
                                     start=True, stop=True)
                    h0 = mpool.tile([128, MM_N], BF)
                    nc.scalar.activation(h0, p0,
                                         mybir.ActivationFunctionType.Relu,
                                         bias=b0[:, 0:1])
                    p1 = qpool.tile([128, MM_N], FP, space="PSUM", tag="p1")
                    nc.tensor.matmul(out=p1, lhsT=w1t, rhs=h0,
                                     start=True, stop=True)
                    h1 = mpool.tile([128, MM_N], BF)
                    nc.scalar.activation(h1, p1,
                                         mybir.ActivationFunctionType.Relu,
                                         bias=b1[:, 0:1])
                    p2 = qpool.tile([128, kper * 64], FP, space="PSUM", tag="p2")
                    for j in range(kper):
                        nc.tensor.matmul(out=p2[:, j * 64:(j + 1) * 64],
                                         lhsT=h1[:, j * 128:(j + 1) * 128],
                                         rhs=w2t, start=True, stop=True)
                    for j in range(kper):
                        kk = cc * kper + j
                        nc.vector.tensor_tensor(
                            out=outt[:, kk * 64:(kk + 1) * 64],
                            in0=p2[:, j * 64:(j + 1) * 64], in1=b2r,
                            op=mybir.AluOpType.add)

                nc.sync.dma_start(
                    out_d[blk * 128 * KC:(blk + 1) * 128 * KC]
                    .rearrange("(p c) f -> p (c f)", p=128),
                    outt)

    nc.compile()
    return nc


# ---------------------------------------------------------------------------
# host-side data prep
# ---------------------------------------------------------------------------

def make_table(planes_list):
    """-> [(H-1)*765, 384] bf16; row (iy*3+pl)*255 + j2."""
    j2 = np.arange(J2N)
    e0 = (j2 - 3) // 4
    e1 = (j2 - 1) // 2
    cols = np.stack([e0, e0 + 1, e0 + 2, e1, e1 + 1, e1 + 2,
                     j2, j2 + 1, 2 * j2, 2 * j2 + 1, 2 * j2 + 2, 2 * j2 + 3],
                    axis=1)                                    # [255, 12]
    tab = np.empty((H - 1, NP, J2N, NCOL, 32), np.float32)
    for sl in range(NCOL):
        s = SLOT_S[sl]
        P = planes_list[s]                                     # [3,16,150,W]
        c = np.clip(cols[:, sl], 0, WS[s] - 1)                 # [255]
        v = P[:, :, :, c]                                      # [3,16,150,255]
        vt = v.transpose(0, 2, 3, 1)                           # [3,150,255,16]
        tab[:, :, :, sl, 0:16] = vt[:, :H - 1].transpose(1, 0, 2, 3)
        tab[:, :, :, sl, 16:32] = (vt[:, 1:] - vt[:, :H - 1]).transpose(1, 0, 2, 3)
    return np.ascontiguousarray(
        tab.reshape((H - 1) * RPY, RW)).astype(ml_dtypes.bfloat16)


def make_cns():
    c = np.zeros((128, NCOL), np.float32)
    c[:] = np.array(SLOT_M, np.float32)
    return c


def bucket_of_t(t):
    ay = np.float32(0.5 * (H - 1))
    iyf = np.clip(t.astype(np.float32) * ay + ay, 0.0, H - 1)
    iy0 = np.minimum(np.floor(iyf), H - 2).astype(np.int64)
    iy0 = np.maximum(iy0, 0)
    return iy0 // YB


def idx_weights(shard, block_buckets, k):
    """shard [L,4] -> wcat [nb,128,13k] fp32, idx16 [nb,128,3k*8] int16."""
    nb = len(block_buckets)
    arr = shard.reshape(nb, 128, k, 4)
    x = arr[..., 0:3]                                          # [nb,128,k,3]
    t = arr[..., 3]
    ay = np.float32(0.5 * (H - 1))
    iyf = np.clip(t * ay + ay, 0.0, H - 1)
    iy0 = np.minimum(np.floor(iyf), H - 2).astype(np.int64)
    wy = (iyf - iy0).astype(np.float32)                        # [nb,128,k]

    ixf = []
    for s in range(NS):
        a = np.float32(0.5 * (WS[s] - 1))
        ixf.append(np.clip(x * a + a, 0.0, WS[s] - 1))         # [nb,128,k,3]
    j2 = np.minimum(np.floor(ixf[2]), WS[2] - 2).astype(np.int64)
    e0 = (j2 - 3) // 4
    e1 = (j2 - 1) // 2
    zbase = [e0, e1, j2, 2 * j2]

    # wcat streams: s-major, plane-minor (z values), then wy
    wcat = np.empty((nb, 128, 13, k), np.float32)
    for s in range(NS):
        z = (ixf[s] - zbase[s]).astype(np.float32)             # [nb,128,k,3]
        wcat[:, :, s * NP:(s + 1) * NP, :] = z.transpose(0, 1, 3, 2)
    wcat[:, :, 12, :] = wy
    wcat = np.ascontiguousarray(wcat.reshape(nb, 128, 13 * k))

    wlo = np.array([win_lo(b) for b in block_buckets], np.int64)
    rowi = ((iy0[..., None] * NP + np.arange(NP)) * J2N + j2
            - (wlo[:, None, None, None] * RPY))                # [nb,128,k,3]
    assert rowi.min() >= 0 and rowi.max() < 32768, (rowi.min(), rowi.max())

    # wrapped layout: j = (pl*k + c)*128 + p; idx16[q, (pl*k+c)*8 + h]
    idx_r = rowi.transpose(0, 1, 3, 2).reshape(nb, 128, NP * k)
    w16 = idx_r.reshape(nb, 8, 16, NP * k).transpose(0, 2, 3, 1)
    w16 = w16.reshape(nb, 16, NP * k * 8)
    w16 = np.broadcast_to(w16[:, None], (nb, 8, 16, NP * k * 8))
    return wcat, np.ascontiguousarray(
        w16.reshape(nb, 128, NP * k * 8)).astype(np.int16)


def bucket_layout(pts, k):
    n = pts.shape[0]
    percore = (n + NCORES - 1) // NCORES
    pb = 128 * k

    shards = [pts[c * percore:(c + 1) * percore] for c in range(NCORES)]
    bkts = [bucket_of_t(sh[:, 3]) for sh in shards]
    order = [np.argsort(b, kind="stable") for b in bkts]
    counts = np.zeros((NCORES, NBKT), np.int64)
    for c in range(NCORES):
        for b in range(NBKT):
            counts[c, b] = int((bkts[c] == b).sum())
    nb_per_bucket = [int(math.ceil(counts[:, b].max() / pb)) for b in range(NBKT)]
    block_buckets = []
    for b in range(NBKT):
        block_buckets += [b] * nb_per_bucket[b]
    nb = len(block_buckets)
    L = nb * pb

    cores = []
    for c in range(NCORES):
        sh, od, bk = shards[c], order[c], bkts[c]
        rows = np.zeros((L, 4), np.float32)
        perm = np.full(L, -1, np.int64)
        pos = 0
        for b in range(NBKT):
            sel = od[bk[od] == b]
            nrows = nb_per_bucket[b] * pb
            rows[pos:pos + len(sel)] = sh[sel]
            tpad = (b * YB + YB // 2) / (0.5 * (H - 1)) - 1.0
            if nrows > len(sel):
                rows[pos + len(sel):pos + nrows, 3] = tpad
            perm[pos:pos + len(sel)] = sel
            pos += nrows
        cores.append((rows, perm))
    return cores, block_buckets, percore


def host_inputs(pts, planes_list, w0, b0, w1, b1, w2, b2, k=K):
    bf = ml_dtypes.bfloat16
    shared = {
        "tab": make_table(planes_list),
        "cns": make_cns(),
        "w0t": np.ascontiguousarray(w0.T).astype(bf),
        "w1t": np.ascontiguousarray(w1.T).astype(bf),
        "w2t": np.ascontiguousarray(w2.T).astype(bf),
        "b0c": np.ascontiguousarray(b0.reshape(128, 1)),
        "b1c": np.ascontiguousarray(b1.reshape(128, 1)),
        "b2r": np.ascontiguousarray(np.broadcast_to(b2.reshape(1, 64), (128, 64))),
    }
    cores, block_buckets, percore = bucket_layout(pts, k)
    in_maps, perms = [], []
    for rows, perm in cores:
        wcat, w16 = idx_weights(rows, block_buckets, k)
        in_maps.append({**shared, "wcat": wcat, "idx16": w16})
        perms.append(perm)
    return in_maps, perms, block_buckets, percore


# ---------------------------------------------------------------------------
# numpy emulation (layout validation without HW)
# ---------------------------------------------------------------------------

def emulate(in_map, block_buckets, k=K):
    bf = ml_dtypes.bfloat16
    nb = len(block_buckets)
    wcat = in_map["wcat"]
    w16 = in_map["idx16"]
    tab = np.asarray(in_map["tab"], dtype=bf).astype(np.float32)
    out = np.empty((nb * 128 * k, 64), np.float32)
    for b in range(nb):
        base = win_lo(block_buckets[b]) * RPY
        idxs = w16[b, :16].reshape(16, NP * k, 8).astype(np.int64)
        rows_g = idxs.transpose(2, 0, 1).reshape(128, NP * k)
        g = tab[base + rows_g].reshape(128, NP, k, NCOL, 2, 16)
        wc = wcat[b].reshape(128, 13, k)
        wy = wc[:, 12]                                     # [128,k]
        feats = np.ones((128, k, 64), np.float32)
        for pl in range(NP):
            for s in range(NS):
                z = wc[:, s * NP + pl]                     # [128,k]
                acc = np.zeros((128, k, 16), np.float32)
                for m in range(SLOT_N[s]):
                    sl = SLOT_OFF[s] + m
                    hat = np.maximum(0, 1 - np.abs(z - m)).astype(bf).astype(np.float32)
                    hdv = (hat * wy).astype(bf).astype(np.float32)
                    term = (g[:, pl, :, sl, 0] * hat[..., None]
                            + g[:, pl, :, sl, 1] * hdv[..., None])
                    acc += term.astype(bf).astype(np.float32)
                feats[:, :, s * 16:(s + 1) * 16] *= acc.astype(bf).astype(np.float32)
        xx = feats.reshape(128 * k, 64)
        h = np.maximum(xx @ np.asarray(in_map["w0t"], dtype=bf).astype(np.float32), 0)
        h = np.maximum(h @ np.asarray(in_map["w1t"], dtype=bf).astype(np.float32), 0)
        y = h @ np.asarray(in_map["w2t"], dtype=bf).astype(np.float32) + in_map["b2r"][0]
        out[b * 128 * k:(b + 1) * 128 * k] = y
    return out


# ---------------------------------------------------------------------------
# entry point
# ---------------------------------------------------------------------------

_CACHE = {}


def kernel(pts, planes_s0, planes_s1, planes_s2, planes_s3,
           w0, b0, w1, b1, w2, b2, _want_trace=False):
    from concourse.bass_utils import run_bass_kernel_spmd

    pts = np.asarray(pts, np.float32)
    planes = [np.asarray(p, np.float32)
              for p in (planes_s0, planes_s1, planes_s2, planes_s3)]
    in_maps, perms, block_buckets, percore = host_inputs(
        pts, planes,
        np.asarray(w0, np.float32), np.asarray(b0, np.float32),
        np.asarray(w1, np.float32), np.asarray(b1, np.float32),
        np.asarray(w2, np.float32), np.asarray(b2, np.float32))

    import time as _t
    key = (tuple(block_buckets), K)
    if key not in _CACHE:
        t0 = _t.time()
        print(f"[kernel] building program nb={len(block_buckets)}", flush=True)
        _CACHE[key] = build_program(block_buckets, K, num_devices=NCORES)
        print(f"[kernel] build done {_t.time()-t0:.1f}s", flush=True)
    nc = _CACHE[key]

    t0 = _t.time()
    print("[kernel] launching on 8 cores", flush=True)
    r = run_bass_kernel_spmd(nc, in_maps, core_ids=list(range(NCORES)),
                             trace=_want_trace)
    print(f"[kernel] run done {_t.time()-t0:.1f}s", flush=True)
    n = pts.shape[0]
    full = np.empty((n, 64), np.float32)
    for c in range(NCORES):
        dev = np.asarray(r.results[c]["out"])
        perm = perms[c]
        valid = perm >= 0
        base = c * percore
        full[base + perm[valid]] = dev[valid]
    if _want_trace:
        return full, r
    return full


# revision 3
# speedup vs baseline: 1.3980x; 1.0236x over previous
"""Trainium2 Bass kernel v3: quad-table gather for K-Planes lookup + MLP.

Key idea: the baseline gathers one 256B delta-form row per
(point, scale, plane) = 12 dma_gather indices per point, and the Q7
SWDGE descriptor generation (~8ns/index) dominates (90% of runtime).

v3 restructures the table so ONE gathered row serves all 4 scales of
one plane: row (iy, pl, j2) holds 12 corner COLUMNS [v(16ch), dy(16ch)]
(y-lerp delta form):
    s0: cols e0,e0+1,e0+2   e0 = (j2-3)//4
    s1: cols e1,e1+1,e1+2   e1 = (j2-1)//2
    s2: cols j2,j2+1
    s3: cols 2j2..2j2+3
where j2 = s2 cell of the point.  The candidate sets provably cover the
cells every scale needs.  x-interp becomes a hat-weighted sum over the
columns: res_s = sum_m relu(1-|z_s-m|) * (v_m + wy*dy_m), z_s = local
fractional coordinate (host-computed).  3 indices/point instead of 12.

bf16 table + bf16 interp + bf16 MLP (rel err ~7e-3 vs 2e-2 budget).
Host precomputes idx16 (wrapped+replicated int16) and the 13-stream
z/wy weights; the device does zero index math.
"""

import math
import numpy as np
import ml_dtypes
from contextlib import ExitStack

import concourse.bass as bass
import concourse.bacc as bacc
import concourse.mybir as mybir
import concourse.tile as tile
from concourse import library_config
from concourse.masks import make_identity

FP = mybir.dt.float32
BF = mybir.dt.bfloat16
I16 = mybir.dt.int16

H = 150
WS = (64, 128, 256, 512)
NP = 3
NS = 4
NCORES = 8
YB = 32
NBKT = (H - 2) // YB + 1          # iy0 in [0,148] -> 5 buckets
J2N = WS[2] - 1                   # 255 j2 values (cells 0..254 + clamp pad)
RPY = NP * J2N                    # rows per iy = 765
NCOL = 12                         # corner columns per row
RW = NCOL * 32                    # row elements (bf16): 384 = 768B
SLOT_S = [0, 0, 0, 1, 1, 1, 2, 2, 3, 3, 3, 3]   # scale of each col slot
SLOT_M = [0, 1, 2, 0, 1, 2, 0, 1, 0, 1, 2, 3]   # m offset of each col slot
SLOT_OFF = [0, 3, 6, 8]                          # first slot of each scale
SLOT_N = [3, 3, 2, 4]                            # cols per scale

K = 16            # point-cols per partition per block (block = 2048 pts)
MM_N = 512


def win_lo(b):
    return b * YB


def win_rows(b):
    return min(H - 1, (b + 1) * YB) - win_lo(b)   # <= 32


# ---------------------------------------------------------------------------
# device program
# ---------------------------------------------------------------------------

def build_program(block_buckets, k: int = K, num_devices: int = 1):
    nc = bacc.Bacc("TRN2", target_bir_lowering=False, debug=False,
                   enable_asserts=False, num_devices=num_devices)

    KC = k
    nb = len(block_buckets)
    L = nb * 128 * KC
    NIDX = NP * KC * 128

    wh_d = nc.dram_tensor("wh", [nb, 128, NP * KC * NCOL * 2], BF,
                          kind="ExternalInput").ap()
    idx_d = nc.dram_tensor("idx16", [nb, 128, NP * KC * 8], I16,
                           kind="ExternalInput").ap()
    tab_d = nc.dram_tensor("tab", [(H - 1) * RPY, RW], BF, kind="ExternalInput").ap()
    w0t_d = nc.dram_tensor("w0t", [64, 128], BF, kind="ExternalInput").ap()
    w1t_d = nc.dram_tensor("w1t", [128, 128], BF, kind="ExternalInput").ap()
    w2t_d = nc.dram_tensor("w2t", [128, 64], BF, kind="ExternalInput").ap()
    b0_d = nc.dram_tensor("b0c", [128, 1], FP, kind="ExternalInput").ap()
    b1_d = nc.dram_tensor("b1c", [128, 1], FP, kind="ExternalInput").ap()
    b2_d = nc.dram_tensor("b2r", [128, 64], FP, kind="ExternalInput").ap()
    out_d = nc.dram_tensor("out", [L, 64], FP, kind="ExternalOutput").ap()

    with tile.TileContext(nc) as tc:
        with ExitStack() as ctx:
            cpool = ctx.enter_context(tc.tile_pool(name="cpool", bufs=1))
            ppool = ctx.enter_context(tc.tile_pool(name="ppool", bufs=2))
            gpool = ctx.enter_context(tc.tile_pool(name="gpool", bufs=2))
            ipool = ctx.enter_context(tc.tile_pool(name="ipool", bufs=2))
            fpool = ctx.enter_context(tc.tile_pool(name="fpool", bufs=2))
            mpool = ctx.enter_context(tc.tile_pool(name="mpool", bufs=2))
            qpool = ctx.enter_context(tc.tile_pool(name="qpool", bufs=2, space="PSUM"))

            nc.gpsimd.load_library(library_config.mlp)

            identf = cpool.tile([128, 128], FP)
            make_identity(nc, identf)
            ident = cpool.tile([128, 128], BF)
            nc.vector.tensor_copy(ident, identf)
            w0t = cpool.tile([64, 128], BF)
            nc.sync.dma_start(w0t, w0t_d)
            w1t = cpool.tile([128, 128], BF)
            nc.sync.dma_start(w1t, w1t_d)
            w2t = cpool.tile([128, 64], BF)
            nc.sync.dma_start(w2t, w2t_d)
            b0 = cpool.tile([128, 1], FP)
            nc.sync.dma_start(b0, b0_d)
            b1 = cpool.tile([128, 1], FP)
            nc.sync.dma_start(b1, b1_d)
            b2r = cpool.tile([128, 64], FP)
            nc.sync.dma_start(b2r, b2_d)

            for blk in range(nb):
                bkt = block_buckets[blk]
                wh = ppool.tile([128, NP * KC * NCOL * 2], BF)
                nc.sync.dma_start(wh, wh_d[blk])
                idx16 = ppool.tile([128, NP * KC * 8], I16)
                nc.sync.dma_start(idx16, idx_d[blk])

                # ---- gather: one row per (pt, plane) ----
                base = win_lo(bkt) * RPY
                wrows = win_rows(bkt) * RPY
                g = gpool.tile([128, NP * KC * RW], BF, tag="g")
                nc.gpsimd.dma_gather(
                    out_ap=g.rearrange("p (c i) -> p c i", i=RW),
                    in_ap=tab_d[base:base + wrows],
                    idxs_ap=idx16,
                    num_idxs=NIDX,
                    num_idxs_reg=NIDX,
                    elem_size=RW,
                    single_packet=False)

                # ---- apply: g *= wh (broadcast over 16 channels) ----
                gv = g.rearrange("p (pl c sh ch) -> p pl c sh ch",
                                 pl=NP, c=KC, ch=16)
                whb = (wh.rearrange("p (pl c sh) -> p pl c sh", pl=NP, c=KC)
                       .unsqueeze(-1).to_broadcast([128, NP, KC, NCOL * 2, 16]))
                nc.vector.tensor_tensor(out=gv, in0=gv, in1=whb,
                                        op=mybir.AluOpType.mult)
                # fold v/dy halves: tcol = g[..., v] + g[..., dy]
                tcol = ipool.tile([128, NP * KC * NCOL * 16], BF)
                tv = tcol.rearrange("p (pl c sl ch) -> p pl c sl ch",
                                    pl=NP, c=KC, ch=16)
                g6 = g.rearrange("p (pl c sl h ch) -> p pl c sl h ch",
                                 pl=NP, c=KC, h=2, ch=16)
                nc.vector.tensor_tensor(out=tv, in0=g6[:, :, :, :, 0, :],
                                        in1=g6[:, :, :, :, 1, :],
                                        op=mybir.AluOpType.add)
                # ragged adds per scale -> res[p, pl, c, 64]
                res = ipool.tile([128, NP * KC * 64], BF)
                rv = res.rearrange("p (pl c f) -> p pl c f", pl=NP, c=KC)
                for s in range(NS):
                    o = SLOT_OFF[s]
                    dst = rv[:, :, :, s * 16:(s + 1) * 16]
                    nc.vector.tensor_tensor(
                        out=dst, in0=tv[:, :, :, o, :], in1=tv[:, :, :, o + 1, :],
                        op=mybir.AluOpType.add)
                    for m in range(2, SLOT_N[s]):
                        nc.vector.tensor_tensor(
                            out=dst, in0=dst, in1=tv[:, :, :, o + m, :],
                            op=mybir.AluOpType.add)
                # plane product -> feats [p, c, 64]
                pp = ipool.tile([128, KC * 64], BF)
                ppv = pp.rearrange("p (c f) -> p c f", f=64)
                nc.vector.tensor_tensor(out=ppv, in0=rv[:, 0],
                                        in1=rv[:, 1],
                                        op=mybir.AluOpType.mult)
                feats = fpool.tile([128, KC * 64], BF)
                nc.vector.tensor_tensor(out=feats.rearrange("p (c f) -> p c f", f=64),
                                        in0=ppv,
                                        in1=rv[:, 2],
                                        op=mybir.AluOpType.mult)
                featsv = feats.rearrange("p (c i) -> p c i", i=64)

                # ---- MLP (bf16 matmuls, fp32 psum) ----
                outt = fpool.tile([128, KC * 64], FP)
                nchunk = (KC * 128) // MM_N
                kper = MM_N // 128
                for cc in range(nchunk):
                    ftp = qpool.tile([64, MM_N], BF, space="PSUM", tag="ftp")
                    for j in range(kper):
                        kk = cc * kper + j
                        nc.tensor.transpose(
                            out=ftp[:, j * 128:(j + 1) * 128],
                            in_=featsv[:, kk, :], identity=ident)
                    fts = mpool.tile([64, MM_N], BF)
                    nc.scalar.activation(fts, ftp,
                                         mybir.ActivationFunctionType.Copy)
                    p0 = qpool.tile([128, MM_N], FP, space="PSUM", tag="p0")
                    nc.tensor.matmul(out=p0, lhsT=w0t, rhs=fts,
                                     start=True, stop=True)
                    h0 = mpool.tile([128, MM_N], BF)
                    nc.scalar.activation(h0, p0,
                                         mybir.ActivationFunctionType.Relu,
                                         bias=b0[:, 0:1])
                    p1 = qpool.tile([128, MM_N], FP, space="PSUM", tag="p1")
                    nc.tensor.matmul(out=p1, lhsT=w1t, rhs=h0,
                                     start=True, stop=True)
                    h1 = mpool.tile([128, MM_N], BF)
                    nc.scalar.activation(h1, p1,
                                         mybir.ActivationFunctionType.Relu,
                                         bias=b1[:, 0:1])
                    p2 = qpool.tile([128, kper * 64], FP, space="PSUM", tag="p2")
                    for j in range(kper):
                        nc.tensor.matmul(out=p2[:, j * 64:(j + 1) * 64],
                                         lhsT=h1[:, j * 128:(j + 1) * 128],
                                         rhs=w2t, start=True, stop=True)
                    for j in range(kper):
                        kk = cc * kper + j
                        nc.vector.tensor_tensor(
                            out=outt[:, kk * 64:(kk + 1) * 64],
                            in0=p2[:, j * 64:(j + 1) * 64], in1=b2r,
                            op=mybir.AluOpType.add)

                nc.sync.dma_start(
                    out_d[blk * 128 * KC:(blk + 1) * 128 * KC]
                    .rearrange("(p c) f -> p (c f)", p=128),
                    outt)

    nc.compile()
    return nc


# ---------------------------------------------------------------------------
# host-side data prep
# ---------------------------------------------------------------------------

def make_table(planes_list):
    """-> [(H-1)*765, 384] bf16; row (iy*3+pl)*255 + j2."""
    j2 = np.arange(J2N)
    e0 = (j2 - 3) // 4
    e1 = (j2 - 1) // 2
    cols = np.stack([e0, e0 + 1, e0 + 2, e1, e1 + 1, e1 + 2,
                     j2, j2 + 1, 2 * j2, 2 * j2 + 1, 2 * j2 + 2, 2 * j2 + 3],
                    axis=1)                                    # [255, 12]
    tab = np.empty((H - 1, NP, J2N, NCOL, 32), np.float32)
    for sl in range(NCOL):
        s = SLOT_S[sl]
        P = planes_list[s]                                     # [3,16,150,W]
        c = np.clip(cols[:, sl], 0, WS[s] - 1)                 # [255]
        v = P[:, :, :, c]                                      # [3,16,150,255]
        vt = v.transpose(0, 2, 3, 1)                           # [3,150,255,16]
        tab[:, :, :, sl, 0:16] = vt[:, :H - 1].transpose(1, 0, 2, 3)
        tab[:, :, :, sl, 16:32] = (vt[:, 1:] - vt[:, :H - 1]).transpose(1, 0, 2, 3)
    return np.ascontiguousarray(
        tab.reshape((H - 1) * RPY, RW)).astype(ml_dtypes.bfloat16)


def make_cns():
    c = np.zeros((128, NCOL), np.float32)
    c[:] = np.array(SLOT_M, np.float32)
    return c


def bucket_of_t(t):
    ay = np.float32(0.5 * (H - 1))
    iyf = np.clip(t.astype(np.float32) * ay + ay, 0.0, H - 1)
    iy0 = np.minimum(np.floor(iyf), H - 2).astype(np.int64)
    iy0 = np.maximum(iy0, 0)
    return iy0 // YB


def idx_weights(shard, block_buckets, k):
    """shard [L,4] -> wh [nb,128,3k*24] bf16, idx16 [nb,128,3k*8] int16.

    wh[b, p, pl, c, slot, 0] = hat  = relu(1 - |z_s - m|)
    wh[b, p, pl, c, slot, 1] = hat * wy
    """
    nb = len(block_buckets)
    arr = shard.reshape(nb, 128, k, 4)
    x = arr[..., 0:3]                                          # [nb,128,k,3]
    t = arr[..., 3]
    ay = np.float32(0.5 * (H - 1))
    iyf = np.clip(t * ay + ay, 0.0, H - 1)
    iy0 = np.minimum(np.floor(iyf), H - 2).astype(np.int64)
    wy = (iyf - iy0).astype(np.float32)                        # [nb,128,k]

    ixf = []
    for s in range(NS):
        a = np.float32(0.5 * (WS[s] - 1))
        ixf.append(np.clip(x * a + a, 0.0, WS[s] - 1))         # [nb,128,k,3]
    j2 = np.minimum(np.floor(ixf[2]), WS[2] - 2).astype(np.int64)
    e0 = (j2 - 3) // 4
    e1 = (j2 - 1) // 2
    zbase = [e0, e1, j2, 2 * j2]

    bf = ml_dtypes.bfloat16
    wh = np.empty((nb, 128, NP, k, NCOL, 2), bf)
    for s in range(NS):
        z = (ixf[s] - zbase[s]).astype(np.float32)             # [nb,128,k,3pl]
        for m in range(SLOT_N[s]):
            hat = np.maximum(0.0, 1.0 - np.abs(z - m)).astype(bf)
            hd = (hat.astype(np.float32) * wy[..., None]).astype(bf)
            sl = SLOT_OFF[s] + m
            wh[:, :, :, :, sl, 0] = hat.transpose(0, 1, 3, 2)
            wh[:, :, :, :, sl, 1] = hd.transpose(0, 1, 3, 2)
    wh = np.ascontiguousarray(wh.reshape(nb, 128, NP * k * NCOL * 2))

    wlo = np.array([win_lo(b) for b in block_buckets], np.int64)
    rowi = ((iy0[..., None] * NP + np.arange(NP)) * J2N + j2
            - (wlo[:, None, None, None] * RPY))                # [nb,128,k,3]
    assert rowi.min() >= 0 and rowi.max() < 32768, (rowi.min(), rowi.max())

    idx_r = rowi.transpose(0, 1, 3, 2).reshape(nb, 128, NP * k)
    w16 = idx_r.reshape(nb, 8, 16, NP * k).transpose(0, 2, 3, 1)
    w16 = w16.reshape(nb, 16, NP * k * 8)
    w16 = np.broadcast_to(w16[:, None], (nb, 8, 16, NP * k * 8))
    return wh, np.ascontiguousarray(
        w16.reshape(nb, 128, NP * k * 8)).astype(np.int16)


def bucket_layout(pts, k):
    n = pts.shape[0]
    percore = (n + NCORES - 1) // NCORES
    pb = 128 * k

    shards = [pts[c * percore:(c + 1) * percore] for c in range(NCORES)]
    bkts = [bucket_of_t(sh[:, 3]) for sh in shards]
    order = [np.argsort(b, kind="stable") for b in bkts]
    counts = np.zeros((NCORES, NBKT), np.int64)
    for c in range(NCORES):
        for b in range(NBKT):
            counts[c, b] = int((bkts[c] == b).sum())
    nb_per_bucket = [int(math.ceil(counts[:, b].max() / pb)) for b in range(NBKT)]
    block_buckets = []
    for b in range(NBKT):
        block_buckets += [b] * nb_per_bucket[b]
    nb = len(block_buckets)
    L = nb * pb

    cores = []
    for c in range(NCORES):
        sh, od, bk = shards[c], order[c], bkts[c]
        rows = np.zeros((L, 4), np.float32)
        perm = np.full(L, -1, np.int64)
        pos = 0
        for b in range(NBKT):
            sel = od[bk[od] == b]
            nrows = nb_per_bucket[b] * pb
            rows[pos:pos + len(sel)] = sh[sel]
            tpad = (b * YB + YB // 2) / (0.5 * (H - 1)) - 1.0
            if nrows > len(sel):
                rows[pos + len(sel):pos + nrows, 3] = tpad
            perm[pos:pos + len(sel)] = sel
            pos += nrows
        cores.append((rows, perm))
    return cores, block_buckets, percore


def host_inputs(pts, planes_list, w0, b0, w1, b1, w2, b2, k=K):
    bf = ml_dtypes.bfloat16
    shared = {
        "tab": make_table(planes_list),
        "w0t": np.ascontiguousarray(w0.T).astype(bf),
        "w1t": np.ascontiguousarray(w1.T).astype(bf),
        "w2t": np.ascontiguousarray(w2.T).astype(bf),
        "b0c": np.ascontiguousarray(b0.reshape(128, 1)),
        "b1c": np.ascontiguousarray(b1.reshape(128, 1)),
        "b2r": np.ascontiguousarray(np.broadcast_to(b2.reshape(1, 64), (128, 64))),
    }
    cores, block_buckets, percore = bucket_layout(pts, k)
    in_maps, perms = [], []
    for rows, perm in cores:
        wh, w16 = idx_weights(rows, block_buckets, k)
        in_maps.append({**shared, "wh": wh, "idx16": w16})
        perms.append(perm)
    return in_maps, perms, block_buckets, percore


# ---------------------------------------------------------------------------
# numpy emulation (layout validation without HW)
# ---------------------------------------------------------------------------

def emulate(in_map, block_buckets, k=K):
    bf = ml_dtypes.bfloat16
    nb = len(block_buckets)
    whm = np.asarray(in_map["wh"]).reshape(nb, 128, NP, k, NCOL, 2)
    w16 = in_map["idx16"]
    tab = np.asarray(in_map["tab"], dtype=bf).astype(np.float32)
    out = np.empty((nb * 128 * k, 64), np.float32)
    for b in range(nb):
        base = win_lo(block_buckets[b]) * RPY
        idxs = w16[b, :16].reshape(16, NP * k, 8).astype(np.int64)
        rows_g = idxs.transpose(2, 0, 1).reshape(128, NP * k)
        g = tab[base + rows_g].reshape(128, NP, k, NCOL, 2, 16)
        whf = whm[b].astype(np.float32)
        feats = np.ones((128, k, 64), np.float32)
        for pl in range(NP):
            for s in range(NS):
                acc = np.zeros((128, k, 16), np.float32)
                for m in range(SLOT_N[s]):
                    sl = SLOT_OFF[s] + m
                    term = (g[:, pl, :, sl, 0] * whf[:, pl, :, sl, 0:1]
                            + g[:, pl, :, sl, 1] * whf[:, pl, :, sl, 1:2])
                    acc += term.astype(bf).astype(np.float32)
                feats[:, :, s * 16:(s + 1) * 16] *= acc.astype(bf).astype(np.float32)
        xx = feats.reshape(128 * k, 64)
        h = np.maximum(xx @ np.asarray(in_map["w0t"], dtype=bf).astype(np.float32), 0)
        h = np.maximum(h @ np.asarray(in_map["w1t"], dtype=bf).astype(np.float32), 0)
        y = h @ np.asarray(in_map["w2t"], dtype=bf).astype(np.float32) + in_map["b2r"][0]
        out[b * 128 * k:(b + 1) * 128 * k] = y
    return out


# ---------------------------------------------------------------------------
# entry point
# ---------------------------------------------------------------------------

_CACHE = {}


def kernel(pts, planes_s0, planes_s1, planes_s2, planes_s3,
           w0, b0, w1, b1, w2, b2, _want_trace=False):
    from concourse.bass_utils import run_bass_kernel_spmd

    pts = np.asarray(pts, np.float32)
    planes = [np.asarray(p, np.float32)
              for p in (planes_s0, planes_s1, planes_s2, planes_s3)]
    in_maps, perms, block_buckets, percore = host_inputs(
        pts, planes,
        np.asarray(w0, np.float32), np.asarray(b0, np.float32),
        np.asarray(w1, np.float32), np.asarray(b1, np.float32),
        np.asarray(w2, np.float32), np.asarray(b2, np.float32))

    import time as _t
    key = (tuple(block_buckets), K)
    if key not in _CACHE:
        t0 = _t.time()
        print(f"[kernel] building program nb={len(block_buckets)}", flush=True)
        _CACHE[key] = build_program(block_buckets, K, num_devices=NCORES)
        print(f"[kernel] build done {_t.time()-t0:.1f}s", flush=True)
    nc = _CACHE[key]

    t0 = _t.time()
    print("[kernel] launching on 8 cores", flush=True)
    r = run_bass_kernel_spmd(nc, in_maps, core_ids=list(range(NCORES)),
                             trace=_want_trace)
    print(f"[kernel] run done {_t.time()-t0:.1f}s", flush=True)
    n = pts.shape[0]
    full = np.empty((n, 64), np.float32)
    for c in range(NCORES):
        dev = np.asarray(r.results[c]["out"])
        perm = perms[c]
        valid = perm >= 0
        base = c * percore
        full[base + perm[valid]] = dev[valid]
    if _want_trace:
        return full, r
    return full


# revision 4
# speedup vs baseline: 1.4167x; 1.0133x over previous
"""Trainium2 Bass kernel v3: quad-table gather for K-Planes lookup + MLP.

Key idea: the baseline gathers one 256B delta-form row per
(point, scale, plane) = 12 dma_gather indices per point, and the Q7
SWDGE descriptor generation (~8ns/index) dominates (90% of runtime).

v3 restructures the table so ONE gathered row serves all 4 scales of
one plane: row (iy, pl, j2) holds 12 corner COLUMNS [v(16ch), dy(16ch)]
(y-lerp delta form):
    s0: cols e0,e0+1,e0+2   e0 = (j2-3)//4
    s1: cols e1,e1+1,e1+2   e1 = (j2-1)//2
    s2: cols j2,j2+1
    s3: cols 2j2..2j2+3
where j2 = s2 cell of the point.  The candidate sets provably cover the
cells every scale needs.  x-interp becomes a hat-weighted sum over the
columns: res_s = sum_m relu(1-|z_s-m|) * (v_m + wy*dy_m), z_s = local
fractional coordinate (host-computed).  3 indices/point instead of 12.

bf16 table + bf16 interp + bf16 MLP (rel err ~7e-3 vs 2e-2 budget).
Host precomputes idx16 (wrapped+replicated int16) and the 13-stream
z/wy weights; the device does zero index math.
"""

import math
import numpy as np
import ml_dtypes
from contextlib import ExitStack

import concourse.bass as bass
import concourse.bacc as bacc
import concourse.mybir as mybir
import concourse.tile as tile
from concourse import library_config
from concourse.masks import make_identity

FP = mybir.dt.float32
BF = mybir.dt.bfloat16
I16 = mybir.dt.int16

H = 150
WS = (64, 128, 256, 512)
NP = 3
NS = 4
NCORES = 8
YB = 42
NBKT = (H - 2) // YB + 1          # iy0 in [0,148] -> 5 buckets
J2N = WS[2] - 1                   # 255 j2 values (cells 0..254 + clamp pad)
RPY = NP * J2N                    # rows per iy = 765
NCOL = 12                         # corner columns per row
RW = NCOL * 32                    # row elements (bf16): 384 = 768B
SLOT_S = [0, 0, 0, 1, 1, 1, 2, 2, 3, 3, 3, 3]   # scale of each col slot
SLOT_M = [0, 1, 2, 0, 1, 2, 0, 1, 0, 1, 2, 3]   # m offset of each col slot
SLOT_OFF = [0, 3, 6, 8]                          # first slot of each scale
SLOT_N = [3, 3, 2, 4]                            # cols per scale

K = 16            # point-cols per partition per block (block = 2048 pts)
MM_N = 512


def win_lo(b):
    return b * YB


def win_rows(b):
    return min(H - 1, (b + 1) * YB) - win_lo(b)   # <= 32


# ---------------------------------------------------------------------------
# device program
# ---------------------------------------------------------------------------

def build_program(block_buckets, k: int = K, num_devices: int = 1):
    nc = bacc.Bacc("TRN2", target_bir_lowering=False, debug=False,
                   enable_asserts=False, num_devices=num_devices)

    KC = k
    nb = len(block_buckets)
    L = nb * 128 * KC
    NIDX = NP * KC * 128

    wh_d = nc.dram_tensor("wh", [nb, 128, NP * KC * NCOL * 2], BF,
                          kind="ExternalInput").ap()
    idx_d = nc.dram_tensor("idx16", [nb, 128, NP * KC * 8], I16,
                           kind="ExternalInput").ap()
    tab_d = nc.dram_tensor("tab", [(H - 1) * RPY, RW], BF, kind="ExternalInput").ap()
    w0t_d = nc.dram_tensor("w0t", [64, 128], BF, kind="ExternalInput").ap()
    w1t_d = nc.dram_tensor("w1t", [128, 128], BF, kind="ExternalInput").ap()
    w2t_d = nc.dram_tensor("w2t", [128, 64], BF, kind="ExternalInput").ap()
    b0_d = nc.dram_tensor("b0c", [128, 1], FP, kind="ExternalInput").ap()
    b1_d = nc.dram_tensor("b1c", [128, 1], FP, kind="ExternalInput").ap()
    b2_d = nc.dram_tensor("b2r", [128, 64], FP, kind="ExternalInput").ap()
    out_d = nc.dram_tensor("out", [L, 64], FP, kind="ExternalOutput").ap()

    with tile.TileContext(nc) as tc:
        with ExitStack() as ctx:
            cpool = ctx.enter_context(tc.tile_pool(name="cpool", bufs=1))
            ppool = ctx.enter_context(tc.tile_pool(name="ppool", bufs=2))
            gpool = ctx.enter_context(tc.tile_pool(name="gpool", bufs=2))
            ipool = ctx.enter_context(tc.tile_pool(name="ipool", bufs=2))
            fpool = ctx.enter_context(tc.tile_pool(name="fpool", bufs=2))
            mpool = ctx.enter_context(tc.tile_pool(name="mpool", bufs=2))
            qpool = ctx.enter_context(tc.tile_pool(name="qpool", bufs=2, space="PSUM"))

            nc.gpsimd.load_library(library_config.mlp)

            identf = cpool.tile([128, 128], FP)
            make_identity(nc, identf)
            ident = cpool.tile([128, 128], BF)
            nc.vector.tensor_copy(ident, identf)
            w0t = cpool.tile([64, 128], BF)
            nc.sync.dma_start(w0t, w0t_d)
            w1t = cpool.tile([128, 128], BF)
            nc.sync.dma_start(w1t, w1t_d)
            w2t = cpool.tile([128, 64], BF)
            nc.sync.dma_start(w2t, w2t_d)
            b0 = cpool.tile([128, 1], FP)
            nc.sync.dma_start(b0, b0_d)
            b1 = cpool.tile([128, 1], FP)
            nc.sync.dma_start(b1, b1_d)
            b2r = cpool.tile([128, 64], FP)
            nc.sync.dma_start(b2r, b2_d)

            for blk in range(nb):
                bkt = block_buckets[blk]
                wh = ppool.tile([128, NP * KC * NCOL * 2], BF)
                nc.sync.dma_start(wh, wh_d[blk])
                idx16 = ppool.tile([128, NP * KC * 8], I16)
                nc.sync.dma_start(idx16, idx_d[blk])

                # ---- gather: one row per (pt, plane) ----
                base = win_lo(bkt) * RPY
                wrows = win_rows(bkt) * RPY
                g = gpool.tile([128, NP * KC * RW], BF, tag="g")
                nc.gpsimd.dma_gather(
                    out_ap=g.rearrange("p (c i) -> p c i", i=RW),
                    in_ap=tab_d[base:base + wrows],
                    idxs_ap=idx16,
                    num_idxs=NIDX,
                    num_idxs_reg=NIDX,
                    elem_size=RW,
                    single_packet=False)

                # ---- apply: g *= wh (broadcast over 16 channels) ----
                gv = g.rearrange("p (pl c sh ch) -> p pl c sh ch",
                                 pl=NP, c=KC, ch=16)
                whb = (wh.rearrange("p (pl c sh) -> p pl c sh", pl=NP, c=KC)
                       .unsqueeze(-1).to_broadcast([128, NP, KC, NCOL * 2, 16]))
                nc.vector.tensor_tensor(out=gv, in0=gv, in1=whb,
                                        op=mybir.AluOpType.mult)
                # fold v/dy halves: tcol = g[..., v] + g[..., dy]
                tcol = ipool.tile([128, NP * KC * NCOL * 16], BF)
                tv = tcol.rearrange("p (pl c sl ch) -> p pl c sl ch",
                                    pl=NP, c=KC, ch=16)
                g6 = g.rearrange("p (pl c sl h ch) -> p pl c sl h ch",
                                 pl=NP, c=KC, h=2, ch=16)
                nc.vector.tensor_tensor(out=tv, in0=g6[:, :, :, :, 0, :],
                                        in1=g6[:, :, :, :, 1, :],
                                        op=mybir.AluOpType.add)
                # ragged adds per scale -> res[p, pl, c, 64]
                res = ipool.tile([128, NP * KC * 64], BF)
                rv = res.rearrange("p (pl c f) -> p pl c f", pl=NP, c=KC)
                for s in range(NS):
                    o = SLOT_OFF[s]
                    dst = rv[:, :, :, s * 16:(s + 1) * 16]
                    nc.vector.tensor_tensor(
                        out=dst, in0=tv[:, :, :, o, :], in1=tv[:, :, :, o + 1, :],
                        op=mybir.AluOpType.add)
                    for m in range(2, SLOT_N[s]):
                        nc.vector.tensor_tensor(
                            out=dst, in0=dst, in1=tv[:, :, :, o + m, :],
                            op=mybir.AluOpType.add)
                # plane product -> feats [p, c, 64]
                pp = ipool.tile([128, KC * 64], BF)
                ppv = pp.rearrange("p (c f) -> p c f", f=64)
                nc.vector.tensor_tensor(out=ppv, in0=rv[:, 0],
                                        in1=rv[:, 1],
                                        op=mybir.AluOpType.mult)
                feats = fpool.tile([128, KC * 64], BF)
                nc.vector.tensor_tensor(out=feats.rearrange("p (c f) -> p c f", f=64),
                                        in0=ppv,
                                        in1=rv[:, 2],
                                        op=mybir.AluOpType.mult)
                featsv = feats.rearrange("p (c i) -> p c i", i=64)

                # ---- MLP (bf16 matmuls, fp32 psum) ----
                outt = fpool.tile([128, KC * 64], FP)
                nchunk = (KC * 128) // MM_N
                kper = MM_N // 128
                for cc in range(nchunk):
                    ftp = qpool.tile([64, MM_N], BF, space="PSUM", tag="ftp")
                    for j in range(kper):
                        kk = cc * kper + j
                        nc.tensor.transpose(
                            out=ftp[:, j * 128:(j + 1) * 128],
                            in_=featsv[:, kk, :], identity=ident)
                    fts = mpool.tile([64, MM_N], BF)
                    nc.scalar.activation(fts, ftp,
                                         mybir.ActivationFunctionType.Copy)
                    p0 = qpool.tile([128, MM_N], FP, space="PSUM", tag="p0")
                    nc.tensor.matmul(out=p0, lhsT=w0t, rhs=fts,
                                     start=True, stop=True)
                    h0 = mpool.tile([128, MM_N], BF)
                    nc.scalar.activation(h0, p0,
                                         mybir.ActivationFunctionType.Relu,
                                         bias=b0[:, 0:1])
                    p1 = qpool.tile([128, MM_N], FP, space="PSUM", tag="p1")
                    nc.tensor.matmul(out=p1, lhsT=w1t, rhs=h0,
                                     start=True, stop=True)
                    h1 = mpool.tile([128, MM_N], BF)
                    nc.scalar.activation(h1, p1,
                                         mybir.ActivationFunctionType.Relu,
                                         bias=b1[:, 0:1])
                    p2 = qpool.tile([128, kper * 64], FP, space="PSUM", tag="p2")
                    for j in range(kper):
                        nc.tensor.matmul(out=p2[:, j * 64:(j + 1) * 64],
                                         lhsT=h1[:, j * 128:(j + 1) * 128],
                                         rhs=w2t, start=True, stop=True)
                    for j in range(kper):
                        kk = cc * kper + j
                        nc.vector.tensor_tensor(
                            out=outt[:, kk * 64:(kk + 1) * 64],
                            in0=p2[:, j * 64:(j + 1) * 64], in1=b2r,
                            op=mybir.AluOpType.add)

                nc.sync.dma_start(
                    out_d[blk * 128 * KC:(blk + 1) * 128 * KC]
                    .rearrange("(p c) f -> p (c f)", p=128),
                    outt)

    nc.compile()
    return nc


# ---------------------------------------------------------------------------
# host-side data prep
# ---------------------------------------------------------------------------

def make_table(planes_list):
    """-> [(H-1)*765, 384] bf16; row (iy*3+pl)*255 + j2."""
    j2 = np.arange(J2N)
    e0 = (j2 - 3) // 4
    e1 = (j2 - 1) // 2
    cols = np.stack([e0, e0 + 1, e0 + 2, e1, e1 + 1, e1 + 2,
                     j2, j2 + 1, 2 * j2, 2 * j2 + 1, 2 * j2 + 2, 2 * j2 + 3],
                    axis=1)                                    # [255, 12]
    tab = np.empty((H - 1, NP, J2N, NCOL, 32), np.float32)
    for sl in range(NCOL):
        s = SLOT_S[sl]
        P = planes_list[s]                                     # [3,16,150,W]
        c = np.clip(cols[:, sl], 0, WS[s] - 1)                 # [255]
        v = P[:, :, :, c]                                      # [3,16,150,255]
        vt = v.transpose(0, 2, 3, 1)                           # [3,150,255,16]
        tab[:, :, :, sl, 0:16] = vt[:, :H - 1].transpose(1, 0, 2, 3)
        tab[:, :, :, sl, 16:32] = (vt[:, 1:] - vt[:, :H - 1]).transpose(1, 0, 2, 3)
    return np.ascontiguousarray(
        tab.reshape((H - 1) * RPY, RW)).astype(ml_dtypes.bfloat16)


def make_cns():
    c = np.zeros((128, NCOL), np.float32)
    c[:] = np.array(SLOT_M, np.float32)
    return c


def bucket_of_t(t):
    ay = np.float32(0.5 * (H - 1))
    iyf = np.clip(t.astype(np.float32) * ay + ay, 0.0, H - 1)
    iy0 = np.minimum(np.floor(iyf), H - 2).astype(np.int64)
    iy0 = np.maximum(iy0, 0)
    return iy0 // YB


def idx_weights(shard, block_buckets, k):
    """shard [L,4] -> wh [nb,128,3k*24] bf16, idx16 [nb,128,3k*8] int16.

    wh[b, p, pl, c, slot, 0] = hat  = relu(1 - |z_s - m|)
    wh[b, p, pl, c, slot, 1] = hat * wy
    """
    nb = len(block_buckets)
    arr = shard.reshape(nb, 128, k, 4)
    x = arr[..., 0:3]                                          # [nb,128,k,3]
    t = arr[..., 3]
    ay = np.float32(0.5 * (H - 1))
    iyf = np.clip(t * ay + ay, 0.0, H - 1)
    iy0 = np.minimum(np.floor(iyf), H - 2).astype(np.int64)
    wy = (iyf - iy0).astype(np.float32)                        # [nb,128,k]

    ixf = []
    for s in range(NS):
        a = np.float32(0.5 * (WS[s] - 1))
        ixf.append(np.clip(x * a + a, 0.0, WS[s] - 1))         # [nb,128,k,3]
    j2 = np.minimum(np.floor(ixf[2]), WS[2] - 2).astype(np.int64)
    e0 = (j2 - 3) // 4
    e1 = (j2 - 1) // 2
    zbase = [e0, e1, j2, 2 * j2]

    bf = ml_dtypes.bfloat16
    wh = np.empty((nb, 128, NP, k, NCOL, 2), bf)
    for s in range(NS):
        z = (ixf[s] - zbase[s]).astype(np.float32)             # [nb,128,k,3pl]
        for m in range(SLOT_N[s]):
            hat = np.maximum(0.0, 1.0 - np.abs(z - m)).astype(bf)
            hd = (hat.astype(np.float32) * wy[..., None]).astype(bf)
            sl = SLOT_OFF[s] + m
            wh[:, :, :, :, sl, 0] = hat.transpose(0, 1, 3, 2)
            wh[:, :, :, :, sl, 1] = hd.transpose(0, 1, 3, 2)
    wh = np.ascontiguousarray(wh.reshape(nb, 128, NP * k * NCOL * 2))

    wlo = np.array([win_lo(b) for b in block_buckets], np.int64)
    rowi = ((iy0[..., None] * NP + np.arange(NP)) * J2N + j2
            - (wlo[:, None, None, None] * RPY))                # [nb,128,k,3]
    assert rowi.min() >= 0 and rowi.max() < 32768, (rowi.min(), rowi.max())

    idx_r = rowi.transpose(0, 1, 3, 2).reshape(nb, 128, NP * k)
    w16 = idx_r.reshape(nb, 8, 16, NP * k).transpose(0, 2, 3, 1)
    w16 = w16.reshape(nb, 16, NP * k * 8)
    w16 = np.broadcast_to(w16[:, None], (nb, 8, 16, NP * k * 8))
    return wh, np.ascontiguousarray(
        w16.reshape(nb, 128, NP * k * 8)).astype(np.int16)


def bucket_layout(pts, k):
    """Globally balance points across cores per bucket (round-robin within
    each bucket) so per-core bucket counts differ by <=1 and block padding
    is minimal.  perm values are GLOBAL point indices."""
    n = pts.shape[0]
    pb = 128 * k
    bkt = bucket_of_t(pts[:, 3])
    order = np.argsort(bkt, kind="stable")          # global, bucket-sorted
    cores_idx = [[] for _ in range(NCORES)]
    nb_per_bucket = []
    pos = 0
    for b in range(NBKT):
        nb_in_b = int((bkt == b).sum())
        sel = order[pos:pos + nb_in_b]
        pos += nb_in_b
        per = [sel[c::NCORES] for c in range(NCORES)]
        mx = max(len(p) for p in per)
        nb_per_bucket.append(int(math.ceil(mx / pb)) if mx else 0)
        for c in range(NCORES):
            cores_idx[c].append(per[c])
    block_buckets = []
    for b in range(NBKT):
        block_buckets += [b] * nb_per_bucket[b]
    nb = len(block_buckets)
    L = nb * pb

    cores = []
    for c in range(NCORES):
        rows = np.zeros((L, 4), np.float32)
        perm = np.full(L, -1, np.int64)
        pos = 0
        for b in range(NBKT):
            sel = cores_idx[c][b]
            nrows = nb_per_bucket[b] * pb
            rows[pos:pos + len(sel)] = pts[sel]
            tpad = (b * YB + YB // 2) / (0.5 * (H - 1)) - 1.0
            if nrows > len(sel):
                rows[pos + len(sel):pos + nrows, 3] = tpad
            perm[pos:pos + len(sel)] = sel
            pos += nrows
        cores.append((rows, perm))
    return cores, block_buckets


def host_inputs(pts, planes_list, w0, b0, w1, b1, w2, b2, k=K):
    bf = ml_dtypes.bfloat16
    shared = {
        "tab": make_table(planes_list),
        "w0t": np.ascontiguousarray(w0.T).astype(bf),
        "w1t": np.ascontiguousarray(w1.T).astype(bf),
        "w2t": np.ascontiguousarray(w2.T).astype(bf),
        "b0c": np.ascontiguousarray(b0.reshape(128, 1)),
        "b1c": np.ascontiguousarray(b1.reshape(128, 1)),
        "b2r": np.ascontiguousarray(np.broadcast_to(b2.reshape(1, 64), (128, 64))),
    }
    cores, block_buckets = bucket_layout(pts, k)
    in_maps, perms = [], []
    for rows, perm in cores:
        wh, w16 = idx_weights(rows, block_buckets, k)
        in_maps.append({**shared, "wh": wh, "idx16": w16})
        perms.append(perm)
    return in_maps, perms, block_buckets


# ---------------------------------------------------------------------------
# numpy emulation (layout validation without HW)
# ---------------------------------------------------------------------------

def emulate(in_map, block_buckets, k=K):
    bf = ml_dtypes.bfloat16
    nb = len(block_buckets)
    whm = np.asarray(in_map["wh"]).reshape(nb, 128, NP, k, NCOL, 2)
    w16 = in_map["idx16"]
    tab = np.asarray(in_map["tab"], dtype=bf).astype(np.float32)
    out = np.empty((nb * 128 * k, 64), np.float32)
    for b in range(nb):
        base = win_lo(block_buckets[b]) * RPY
        idxs = w16[b, :16].reshape(16, NP * k, 8).astype(np.int64)
        rows_g = idxs.transpose(2, 0, 1).reshape(128, NP * k)
        g = tab[base + rows_g].reshape(128, NP, k, NCOL, 2, 16)
        whf = whm[b].astype(np.float32)
        feats = np.ones((128, k, 64), np.float32)
        for pl in range(NP):
            for s in range(NS):
                acc = np.zeros((128, k, 16), np.float32)
                for m in range(SLOT_N[s]):
                    sl = SLOT_OFF[s] + m
                    term = (g[:, pl, :, sl, 0] * whf[:, pl, :, sl, 0:1]
                            + g[:, pl, :, sl, 1] * whf[:, pl, :, sl, 1:2])
                    acc += term.astype(bf).astype(np.float32)
                feats[:, :, s * 16:(s + 1) * 16] *= acc.astype(bf).astype(np.float32)
        xx = feats.reshape(128 * k, 64)
        h = np.maximum(xx @ np.asarray(in_map["w0t"], dtype=bf).astype(np.float32), 0)
        h = np.maximum(h @ np.asarray(in_map["w1t"], dtype=bf).astype(np.float32), 0)
        y = h @ np.asarray(in_map["w2t"], dtype=bf).astype(np.float32) + in_map["b2r"][0]
        out[b * 128 * k:(b + 1) * 128 * k] = y
    return out


# ---------------------------------------------------------------------------
# entry point
# ---------------------------------------------------------------------------

_CACHE = {}


def kernel(pts, planes_s0, planes_s1, planes_s2, planes_s3,
           w0, b0, w1, b1, w2, b2, _want_trace=False):
    from concourse.bass_utils import run_bass_kernel_spmd

    pts = np.asarray(pts, np.float32)
    planes = [np.asarray(p, np.float32)
              for p in (planes_s0, planes_s1, planes_s2, planes_s3)]
    in_maps, perms, block_buckets = host_inputs(
        pts, planes,
        np.asarray(w0, np.float32), np.asarray(b0, np.float32),
        np.asarray(w1, np.float32), np.asarray(b1, np.float32),
        np.asarray(w2, np.float32), np.asarray(b2, np.float32))

    import time as _t
    key = (tuple(block_buckets), K)
    if key not in _CACHE:
        t0 = _t.time()
        print(f"[kernel] building program nb={len(block_buckets)}", flush=True)
        _CACHE[key] = build_program(block_buckets, K, num_devices=NCORES)
        print(f"[kernel] build done {_t.time()-t0:.1f}s", flush=True)
    nc = _CACHE[key]

    t0 = _t.time()
    print("[kernel] launching on 8 cores", flush=True)
    r = run_bass_kernel_spmd(nc, in_maps, core_ids=list(range(NCORES)),
                             trace=_want_trace)
    print(f"[kernel] run done {_t.time()-t0:.1f}s", flush=True)
    n = pts.shape[0]
    full = np.empty((n, 64), np.float32)
    for c in range(NCORES):
        dev = np.asarray(r.results[c]["out"])
        perm = perms[c]
        valid = perm >= 0
        full[perm[valid]] = dev[valid]
    if _want_trace:
        return full, r
    return full
